# revision 37
# baseline (speedup 1.0000x reference)
"""v5: query-compacted + key-compacted sparse causal attention.

Sharding: 8 cores = 4 batches x 2 key-parity shards.  Host compacts BOTH
the live queries (q_mask kills ~50% of output rows -> never computed) and
the live keys (v_mask, split even/odd between the 2 cores of a batch).

Device per core:
  scores  s[kb] = kt[:,kb]T @ qt[:,sb]    (f32r, out [128k, W])
  exp     p = exp(s)                       (Act, bf16 out)
  causal  p *= (col_iota >= jthr)          (stt on DVE; jthr precomputed on
                                            host in column-index space)
  PV      o[q, 65] += p_subT @ vp[kb]      (bf16 moving, out free = 65)
Output per superblock: [q, 2*65] = (v-dot | l); host sums the 2 parity
cores, divides by l, scatters to live rows.

Superblocks are 256 query-columns except the last, which is trimmed to a
64-multiple to avoid exp'ing padding.  PE is warmed up with dummy matmuls
during the DMA fill so real scores run at full clock.  PV is deferred two
groups so the Act engine streams without waiting on masks/PV.
"""

import sys
from collections import deque

import numpy as np

try:
    import concourse  # noqa: F401
except ImportError:  # pragma: no cover
    sys.path.insert(0, "/opt/trn_rl_repo")

import ml_dtypes

B, T, D = 4, 4096, 64
NCORES = 8
QSB = 256
KB = 128
VW = 65
GCOLS = 1024          # max score/exp group width (PSUM tile cols)
PIPE = 2
NWARM = 10
SBUFS = 3             # score PSUM buffers
TAIL1 = True          # final group is a single key block
DMA_ORDER = "A"       # input DMA schedule variant
HD2 = False           # split the head DMA per superblock
NEG_BIG = 1e9

_compiled = {}
_last_plan = None


def _build_nc(key):
    import concourse.bass as bass
    import concourse.mybir as mybir
    import concourse.tile as tile
    from concourse import bacc

    widths, blocks, slots = key
    nsb = len(widths)
    qoff = [0]
    for w_ in widths:
        qoff.append(qoff[-1] + w_)
    Tqpad = qoff[-1]
    f32 = mybir.dt.float32
    f32r = mybir.dt.float32r
    bf16 = mybir.dt.bfloat16
    nb_tot = blocks[-1]
    ns_tot = sum(len(s) for s in slots)
    maxW = max(widths)
    thw = maxW + max(1, ns_tot)         # col-iota | per-slot thresholds
    nsubs = [-(-w_ // KB) for w_ in widths]
    ooff = [0]
    for ns_ in nsubs:
        ooff.append(ooff[-1] + ns_ * VW)

    # head chunks: superblock 0 (and 1) keys+queries land in early DMAs
    bA = blocks[min(1, nsb - 1)]
    kA = bA * KB
    qA = qoff[min(2, nsb)]
    kh = blocks[0] if (HD2 and nsb > 1) else bA
    qh = qoff[1] if (HD2 and nsb > 1) else qA
    hd2w = (bA - kh) * KB + (qA - qh)

    nc = bacc.Bacc(None, target_bir_lowering=False, debug=False)
    hd_d = nc.declare_dram_parameter("hd", [D, kh * KB + qh], f32r,
                                     isOutput=False)
    hd2_d = (nc.declare_dram_parameter("hd2", [D, hd2w], f32r,
                                       isOutput=False) if hd2w else None)
    qt_d = (nc.declare_dram_parameter("qt", [D, Tqpad - qA], f32r,
                                      isOutput=False)
            if Tqpad > qA else None)
    kt_d = (nc.declare_dram_parameter("kt", [D, (nb_tot - bA) * KB], f32r,
                                      isOutput=False)
            if nb_tot > bA else None)
    vp_d = nc.declare_dram_parameter("vp", [KB, nb_tot * VW], bf16,
                                     isOutput=False)
    th_d = nc.declare_dram_parameter("th", [KB, thw], f32, isOutput=False)
    o_d = nc.declare_dram_parameter("o", [KB, ooff[-1]], f32,
                                    isOutput=True)

    thcol = {}
    c = 0
    for i in range(nsb):
        for (kb, _chi) in slots[i]:
            thcol[(i, kb)] = maxW + c
            c += 1

    with tile.TileContext(nc) as tc:
        with (
            tc.tile_pool(name="const", bufs=1) as cpool,
            tc.tile_pool(name="pt", bufs=6) as ppool,
            tc.tile_pool(name="ob", bufs=3) as obpool,
            tc.tile_pool(name="ps", bufs=SBUFS, space=bass.MemorySpace.PSUM) as spool,
            tc.tile_pool(name="po", bufs=2, space=bass.MemorySpace.PSUM) as opool,
        ):
            hd = cpool.tile([D, kh * KB + qh], f32r)
            hd2 = (cpool.tile([D, hd2w], f32r, name="hd2")
                   if hd2_d is not None else None)
            qt = (cpool.tile([D, Tqpad - qA], f32r, name="qt")
                  if qt_d is not None else None)
            kt = (cpool.tile([D, (nb_tot - bA) * KB], f32r, name="kt")
                  if kt_d is not None else None)
            vp = cpool.tile([KB, nb_tot * VW], bf16)
            th = cpool.tile([KB, thw], f32)
            warm = cpool.tile([KB, QSB], bf16)

            def ktsl(kb):
                if kb < kh:
                    return hd[:, kb * KB:(kb + 1) * KB]
                if kb < bA:
                    return hd2[:, (kb - kh) * KB:(kb - kh + 1) * KB]
                return kt[:, (kb - bA) * KB:(kb - bA + 1) * KB]

            def qtsl(i):
                if qoff[i + 1] <= qh:
                    return hd[:, kh * KB + qoff[i]:kh * KB + qoff[i + 1]]
                if qoff[i + 1] <= qA:
                    k0 = (bA - kh) * KB
                    return hd2[:, k0 + qoff[i] - qh:k0 + qoff[i + 1] - qh]
                return qt[:, qoff[i] - qA:qoff[i + 1] - qA]

            # PE warm-up: matmuls on a memset tile while input DMAs stream,
            # so the PE p-state ramps to full clock before real scores.
            nc.vector.memset(warm[:], 0.0)
            for _ in range(NWARM):
                sw = spool.tile([KB, GCOLS], f32, name="swarm", tag="s")
                nc.tensor.matmul(sw[:, 0:QSB], warm[:, 0:KB], warm[:, 0:QSB],
                                 start=True, stop=True)

            # DMA order: head chunks first, then the rest in roughly the
            # order compute consumes them.
            nc.sync.dma_start(hd[:], hd_d[:])
            if hd2_d is not None:
                nc.sync.dma_start(hd2[:], hd2_d[:])
            if DMA_ORDER == "A":
                if kt_d is not None:
                    nc.sync.dma_start(kt[:], kt_d[:])
                qM = qoff[min(5, nsb)]
                if qt_d is not None and qM > qA:
                    nc.sync.dma_start(qt[:, 0:qM - qA], qt_d[:, 0:qM - qA])
                nc.sync.dma_start(vp[:], vp_d[:])
                nc.sync.dma_start(th[:], th_d[:])
                if qt_d is not None and Tqpad > qM:
                    nc.sync.dma_start(qt[:, qM - qA:], qt_d[:, qM - qA:])
            else:
                qM = qoff[min(4, nsb)]
                if qt_d is not None and qM > qA:
                    nc.sync.dma_start(qt[:, 0:qM - qA], qt_d[:, 0:qM - qA])
                kM = (blocks[min(3, nsb - 1)] - bA) * KB
                if kt_d is not None and kM > 0:
                    nc.sync.dma_start(kt[:, 0:kM], kt_d[:, 0:kM])
                nc.sync.dma_start(th[:], th_d[:])
                nc.sync.dma_start(vp[:], vp_d[:])
                if kt_d is not None and kM < (nb_tot - bA) * KB:
                    nc.sync.dma_start(kt[:, kM:], kt_d[:, kM:])
                if qt_d is not None and Tqpad > qM:
                    nc.sync.dma_start(qt[:, qM - qA:], qt_d[:, qM - qA:])

            # (superblock, block list, is-last-group): masked (causal
            # boundary) blocks go in the EARLIEST group so the last group's
            # PV never waits on a mask - the superblock's output chain
            # starts right after its final exp call.
            items = []
            for i in range(nsb):
                nb = blocks[i]
                masked = [kb for (kb, _chi) in slots[i]]
                clean = [kb for kb in range(nb) if kb not in masked]
                order_kbs = masked + clean
                gmax = max(1, GCOLS // widths[i])
                if (TAIL1 and i == nsb - 1 and masked and clean
                        and len(masked) <= gmax):
                    # last superblock: [masked | clean chunks | one clean]
                    # so every link of the closing chain starts early
                    sizes = [len(masked)]
                    rest = len(clean) - 1
                    while rest > 0:
                        take = min(gmax, rest)
                        sizes.append(take)
                        rest -= take
                    sizes.append(1)
                else:
                    tail1 = TAIL1 and i == nsb - 1 and nb > 1
                    nbb = nb - 1 if tail1 else nb
                    ng = -(-nbb // gmax)
                    bsz, rem = divmod(nbb, ng)
                    sizes = [bsz + 1] * rem + [bsz] * (ng - rem)
                    if tail1:
                        sizes.append(1)
                kb0 = 0
                for gi, sz in enumerate(sizes):
                    items.append((i, tuple(order_kbs[kb0:kb0 + sz]),
                                  gi == len(sizes) - 1))
                    kb0 += sz

            o_ps = {}
            started = set()
            pending = deque()

            maxsub = max(nsubs)

            def emit_pv(i, kbs, last, p):
                w_ = widths[i]
                nsub = nsubs[i]
                for j, kb in enumerate(kbs):
                    for sub in range(nsub):
                        pw = min(KB, w_ - sub * KB)
                        nc.tensor.matmul(
                            o_ps[i][0:pw, sub * VW:(sub + 1) * VW],
                            p[:, j * w_ + sub * KB:j * w_ + sub * KB + pw],
                            vp[:, kb * VW:(kb + 1) * VW],
                            # start zeroes the whole 2KB zero-region (bank):
                            # one accumulation group per o tile
                            start=(i not in started and j == 0 and sub == 0),
                            stop=(last and j == len(kbs) - 1
                                  and sub == nsub - 1),
                        )
                started.add(i)
                if last:
                    ob = obpool.tile([KB, maxsub * VW], f32, name=f"ob{i}",
                                     tag="ob")
                    nfull = w_ // KB
                    if nfull:
                        cw = nfull * VW
                        nc.vector.tensor_copy(ob[:, 0:cw], o_ps[i][:, 0:cw])
                        nc.sync.dma_start(
                            o_d[:, ooff[i]:ooff[i] + cw], ob[:, 0:cw])
                    if nfull < nsub:
                        # trailing sub-block covers < 128 query rows: touch
                        # only the initialized partition range
                        pw = w_ - nfull * KB
                        cl = slice(nfull * VW, (nfull + 1) * VW)
                        dl = slice(ooff[i] + nfull * VW,
                                   ooff[i] + (nfull + 1) * VW)
                        nc.vector.tensor_copy(ob[0:pw, cl],
                                              o_ps[i][0:pw, cl])
                        nc.sync.dma_start(o_d[0:pw, dl], ob[0:pw, cl])
                    del o_ps[i]

            for it, (i, kbs, last) in enumerate(items):
                w_ = widths[i]
                if i not in o_ps:
                    o_ps[i] = opool.tile([KB, maxsub * VW], f32,
                                         name=f"oacc{i}", tag="oacc")
                sz = len(kbs)
                s = spool.tile([KB, GCOLS], f32, tag="s")
                for j, kb in enumerate(kbs):
                    nc.tensor.matmul(
                        s[:, j * w_:(j + 1) * w_],
                        ktsl(kb),
                        qtsl(i),
                        start=True, stop=True,
                    )
                # drain the PV pipeline early near the end so the final
                # output DMA chains start during the last exp calls
                pipe = (PIPE if it < len(items) - PIPE - 1
                        else max(0, len(items) - 2 - it))
                while len(pending) > pipe:
                    emit_pv(*pending.popleft())
                p = ppool.tile([KB, GCOLS], bf16)
                nc.scalar.activation(
                    p[:, 0:sz * w_], s[:, 0:sz * w_],
                    mybir.ActivationFunctionType.Exp,
                )
                for (kb, chi) in slots[i]:
                    if kb in kbs:
                        j = kbs.index(kb)
                        col = thcol[(i, kb)]
                        nc.vector.scalar_tensor_tensor(
                            p[:, j * w_:j * w_ + chi],
                            th[:, 0:chi],
                            th[:, col:col + 1],
                            p[:, j * w_:j * w_ + chi],
                            op0=mybir.AluOpType.is_ge,
                            op1=mybir.AluOpType.mult,
                        )
                pending.append((i, kbs, last, p))
            while pending:
                emit_pv(*pending.popleft())

    nc.compile()
    return nc


def _get_nc(key):
    if key not in _compiled:
        _compiled[key] = _build_nc(key)
    return _compiled[key]


def _host_inputs(query, value, keys, q_mask, v_mask, scale):
    global _last_plan
    scale = np.float32(scale)
    q = np.asarray(query, np.float32)
    v = np.asarray(value, np.float32)
    k = np.asarray(keys, np.float32)
    qm = np.asarray(q_mask).astype(bool)
    vm = np.asarray(v_mask).astype(bool)

    Lqs = [np.flatnonzero(qm[b]) for b in range(B)]
    nlqs = [len(x) for x in Lqs]
    maxq = max(max(nlqs), 64)

    lives = []
    for c in range(NCORES):
        b, par = c // 2, c % 2
        lives.append(np.flatnonzero(vm[b])[par::2])

    # per-core packed keys: only those visible to some live query
    packed = []
    for c in range(NCORES):
        b = c // 2
        live = lives[c]
        ncnt = int(np.searchsorted(live, Lqs[b][-1] + 1)) if nlqs[b] else 0
        packed.append(live[:ncnt])

    # DP over superblock boundaries (64-col granularity): minimize exp work
    # = sum(blocks_i * W_i) + per-call overhead, where blocks_i is the
    # max-over-cores key-block count at the superblock's top query.
    P = -(-maxq // 64)
    nbat = [0] * (P + 1)
    for pos in range(1, P + 1):
        mx = 1
        for c in range(NCORES):
            b = c // 2
            nlq = nlqs[b]
            if nlq == 0:
                continue
            top = Lqs[b][min(pos * 64, nlq) - 1]
            cn = int(np.searchsorted(packed[c], top + 1))
            mx = max(mx, -(-cn // KB))
        nbat[pos] = mx
    OVH = 222               # act per-call overhead in column-equivalents
    INF = float("inf")
    dp = [INF] * (P + 1)
    prev = [0] * (P + 1)
    dp[0] = 0.0
    for j1 in range(1, P + 1):
        # score matmuls must fit a 2KB PSUM bank: W in {256, 512}, and the
        # final (partial) superblock may be {64, 128} as well
        deltas = (4, 8) if j1 < P else (1, 2, 4, 8)
        for dlt in deltas:
            j0 = j1 - dlt
            if j0 < 0 or dp[j0] == INF:
                continue
            w_ = dlt * 64
            nb = nbat[j1]
            ng = -(-nb // max(1, GCOLS // w_))
            cost = dp[j0] + nb * w_ + OVH * ng
            if cost < dp[j1]:
                dp[j1] = cost
                prev[j1] = j0
    widths = []
    j = P
    while j > 0:
        widths.append((j - prev[j]) * 64)
        j = prev[j]
    widths = tuple(reversed(widths))
    nsb = len(widths)
    qoff = [0]
    for w_ in widths:
        qoff.append(qoff[-1] + w_)
    Tqpad = qoff[-1]

    # per-core, per-superblock base/top q_orig and key count
    base = np.full((NCORES, nsb), -1, np.int64)      # -1: no real cols
    cnt = np.zeros((NCORES, nsb), np.int64)
    for c in range(NCORES):
        b = c // 2
        Lq, nlq = Lqs[b], nlqs[b]
        for i in range(nsb):
            j0 = qoff[i]
            if j0 < nlq:
                base[c, i] = Lq[j0]
                top = Lq[min(qoff[i + 1], nlq) - 1]
                cnt[c, i] = np.searchsorted(packed[c], top + 1)
            else:
                cnt[c, i] = cnt[c, i - 1] if i else 0

    blocks = []
    for i in range(nsb):
        nb = max(1, int(max(-(-cnt[c, i] // KB) for c in range(NCORES))))
        if blocks:
            nb = max(nb, blocks[-1])
        blocks.append(nb)
    nb_tot = blocks[-1]
    npad = nb_tot * KB

    # causal-boundary slots: walk blocks from the top; a block is clean when
    # for every core all its real keys are <= that core's superblock base.
    slots = []
    for i in range(nsb):
        sl = []
        for kb in range(blocks[i] - 1, -1, -1):
            allvis = True
            chi = 0
            for c in range(NCORES):
                b = c // 2
                seg = packed[c][kb * KB:(kb + 1) * KB]
                if seg.size == 0 or base[c, i] < 0:
                    continue
                kmax = int(seg[-1])
                if kmax > base[c, i]:
                    allvis = False
                Lq_sb = Lqs[b][qoff[i]:min(qoff[i + 1], nlqs[b])]
                chi = max(chi, int(np.searchsorted(Lq_sb, kmax)))
            if chi > 0:
                sl.append((kb, chi))
            if allvis:
                break
        slots.append(tuple(reversed(sl)))
    slots = tuple(slots)
    key = (widths, tuple(blocks), slots)

    ns_tot = sum(len(s) for s in slots)
    maxW = max(widths)
    thw = maxW + max(1, ns_tot)
    nsubs = [-(-w_ // KB) for w_ in widths]
    ooff = [0]
    for ns_ in nsubs:
        ooff.append(ooff[-1] + ns_ * VW)
    bA = blocks[min(1, nsb - 1)]
    kA = bA * KB
    qA = qoff[min(2, nsb)]
    kh = blocks[0] if (HD2 and nsb > 1) else bA
    qh = qoff[1] if (HD2 and nsb > 1) else qA
    in_maps = []
    for c in range(NCORES):
        b = c // 2
        pk = packed[c]
        ncnt = len(pk)
        k_orig = np.full(npad, T, np.int64)
        k_orig[:ncnt] = pk
        kc = np.zeros((npad, D), np.float32)
        kc[:ncnt] = k[b][pk]
        vc = np.zeros((npad, VW), np.float32)
        vc[:ncnt, :D] = v[b][pk]
        vc[:ncnt, D] = 1.0
        qt = np.zeros((D, Tqpad), np.float32)
        if nlqs[b]:
            qt[:, :nlqs[b]] = (q[b][Lqs[b]] * scale).T
        kt = np.ascontiguousarray(kc.T)
        vp = np.ascontiguousarray(
            vc.reshape(nb_tot, KB, VW).transpose(1, 0, 2).reshape(KB, -1)
        ).astype(ml_dtypes.bfloat16)
        th = np.zeros((KB, thw), np.float32)
        th[:, :maxW] = np.arange(maxW, dtype=np.float32)[None, :]
        col = maxW
        for i in range(nsb):
            Lq_sb = Lqs[b][qoff[i]:min(qoff[i + 1], nlqs[b])]
            for (kb, _chi) in slots[i]:
                # threshold in column-index space: col kept iff its index
                # >= #cols with q_orig < k_orig  (q_orig >= k_orig)
                th[:, col] = np.searchsorted(
                    Lq_sb, k_orig[kb * KB:(kb + 1) * KB]).astype(np.float32)
                col += 1
        im = {"hd": np.ascontiguousarray(
                  np.concatenate([kt[:, :kh * KB], qt[:, :qh]], axis=1)),
              "vp": vp, "th": th}
        if (bA - kh) * KB + (qA - qh) > 0:
            im["hd2"] = np.ascontiguousarray(
                np.concatenate([kt[:, kh * KB:kA], qt[:, qh:qA]], axis=1))
        if Tqpad > qA:
            im["qt"] = np.ascontiguousarray(qt[:, qA:])
        if nb_tot > bA:
            im["kt"] = np.ascontiguousarray(kt[:, kA:])
        in_maps.append(im)

    _last_plan = {"Lqs": Lqs, "lives": lives, "packed": packed,
                  "blocks": blocks, "slots": slots, "nsb": nsb,
                  "widths": widths, "qoff": qoff, "ooff": ooff,
                  "nsubs": nsubs,
                  "Tqpad": Tqpad, "base": base, "cnt": cnt}
    return in_maps, key


def _host_gather(results, query, value, keys, q_mask, v_mask, scale):
    q = np.asarray(query, np.float32)
    v = np.asarray(value, np.float32)
    k = np.asarray(keys, np.float32)
    vm = np.asarray(v_mask).astype(bool)
    scale = np.float32(scale)
    plan = _last_plan
    nsb = plan["nsb"]
    widths, qoff = plan["widths"], plan["qoff"]

    out = np.zeros((B, T, D), np.float32)
    for b in range(B):
        Lq = plan["Lqs"][b]
        nlq = len(Lq)
        if nlq == 0:
            continue
        osum = results[2 * b]["o"].astype(np.float32) \
            + results[2 * b + 1]["o"].astype(np.float32)
        ooff = plan["ooff"]
        arr = np.empty((qoff[-1], VW), np.float32)
        for i in range(nsb):
            for sub in range(plan["nsubs"][i]):
                pw = min(KB, widths[i] - sub * KB)
                cols = slice(ooff[i] + sub * VW, ooff[i] + (sub + 1) * VW)
                r0 = qoff[i] + sub * KB
                arr[r0:r0 + pw] = osum[0:pw, cols]
        arr = arr[:nlq]
        l = arr[:, D]
        rows = arr[:, :D] / np.where(l > 0, l, 1.0)[:, None]
        nz = np.flatnonzero(vm[b])
        first = nz[0] if nz.size else T
        fix = np.flatnonzero(Lq < first)
        if fix.size:
            rr = Lq[fix]
            s = ((q[b, rr] @ k[b].T) * scale).astype(np.float32)
            s = s - np.float32(NEG_BIG)
            s = s.astype(np.float64)
            s -= s.max(axis=1, keepdims=True)
            p = np.exp(s)
            p /= p.sum(axis=1, keepdims=True)
            rows[fix] = (p @ v[b].astype(np.float64)).astype(np.float32)
        out[b][Lq] = rows
    return out


def kernel(**inputs):
    from concourse.bass_utils import run_bass_kernel_spmd

    in_maps, key = _host_inputs(**inputs)
    nc = _get_nc(key)
    res = run_bass_kernel_spmd(nc, in_maps, list(range(NCORES))).results
    return _host_gather(res, **inputs)


# revision 41
# speedup vs baseline: 1.0379x; 1.0379x over previous
"""v5: query-compacted + key-compacted sparse causal attention.

Sharding: 8 cores = 4 batches x 2 key-parity shards.  Host compacts BOTH
the live queries (q_mask kills ~50% of output rows -> never computed) and
the live keys (v_mask, split even/odd between the 2 cores of a batch).

Device per core:
  scores  s[kb] = kt[:,kb]T @ qt[:,sb]    (f32r, out [128k, W])
  exp     p = exp(s)                       (Act, bf16 out)
  causal  p *= (col_iota >= jthr)          (stt on DVE; jthr precomputed on
                                            host in column-index space)
  PV      o[q, 65] += p_subT @ vp[kb]      (bf16 moving, out free = 65)
Output per superblock: [q, 2*65] = (v-dot | l); host sums the 2 parity
cores, divides by l, scatters to live rows.

Superblocks are 256 query-columns except the last, which is trimmed to a
64-multiple to avoid exp'ing padding.  PE is warmed up with dummy matmuls
during the DMA fill so real scores run at full clock.  PV is deferred two
groups so the Act engine streams without waiting on masks/PV.
"""

import sys
from collections import deque

import numpy as np

try:
    import concourse  # noqa: F401
except ImportError:  # pragma: no cover
    sys.path.insert(0, "/opt/trn_rl_repo")

import ml_dtypes

B, T, D = 4, 4096, 64
NCORES = 8
QSB = 256
KB = 128
VW = 65
GCOLS = 1024          # max score/exp group width (PSUM tile cols)
PIPE = 2
NWARM = 10
SBUFS = 3             # score PSUM buffers
TAIL1 = True          # final group is a single key block
DMA_ORDER = "A"       # input DMA schedule variant
HD2 = False           # split the head DMA per superblock
NEG_BIG = 1e9

_compiled = {}
_last_plan = None


def _build_nc(key):
    import concourse.bass as bass
    import concourse.mybir as mybir
    import concourse.tile as tile
    from concourse import bacc

    widths, blocks, slots = key
    nsb = len(widths)
    qoff = [0]
    for w_ in widths:
        qoff.append(qoff[-1] + w_)
    Tqpad = qoff[-1]
    f32 = mybir.dt.float32
    f32r = mybir.dt.float32r
    bf16 = mybir.dt.bfloat16
    nb_tot = blocks[-1]
    ns_tot = sum(len(s) for s in slots)
    maxW = max(widths)
    thw = maxW + max(1, ns_tot)         # col-iota | per-slot thresholds
    nsubs = [-(-w_ // KB) for w_ in widths]
    ooff = [0]
    for ns_ in nsubs:
        ooff.append(ooff[-1] + ns_ * VW)

    # head chunks: superblock 0 (and 1) keys+queries land in early DMAs
    bA = blocks[min(1, nsb - 1)]
    kA = bA * KB
    qA = qoff[min(2, nsb)]
    kh = blocks[0] if (HD2 and nsb > 1) else bA
    qh = qoff[1] if (HD2 and nsb > 1) else qA
    hd2w = (bA - kh) * KB + (qA - qh)

    nc = bacc.Bacc(None, target_bir_lowering=False, debug=False)
    hd_d = nc.declare_dram_parameter("hd", [D, kh * KB + qh], f32r,
                                     isOutput=False)
    hd2_d = (nc.declare_dram_parameter("hd2", [D, hd2w], f32r,
                                       isOutput=False) if hd2w else None)
    qt_d = (nc.declare_dram_parameter("qt", [D, Tqpad - qA], f32r,
                                      isOutput=False)
            if Tqpad > qA else None)
    kt_d = (nc.declare_dram_parameter("kt", [D, (nb_tot - bA) * KB], f32r,
                                      isOutput=False)
            if nb_tot > bA else None)
    vp_d = nc.declare_dram_parameter("vp", [KB, nb_tot * VW], bf16,
                                     isOutput=False)
    th_d = nc.declare_dram_parameter("th", [KB, thw], f32, isOutput=False)
    o_d = nc.declare_dram_parameter("o", [KB, ooff[-1]], f32,
                                    isOutput=True)

    thcol = {}
    c = 0
    for i in range(nsb):
        for (kb, _chi) in slots[i]:
            thcol[(i, kb)] = maxW + c
            c += 1

    with tile.TileContext(nc) as tc:
        with (
            tc.tile_pool(name="const", bufs=1) as cpool,
            tc.tile_pool(name="pt", bufs=6) as ppool,
            tc.tile_pool(name="ob", bufs=3) as obpool,
            tc.tile_pool(name="ps", bufs=SBUFS, space=bass.MemorySpace.PSUM) as spool,
            tc.tile_pool(name="po", bufs=2, space=bass.MemorySpace.PSUM) as opool,
        ):
            hd = cpool.tile([D, kh * KB + qh], f32r)
            hd2 = (cpool.tile([D, hd2w], f32r, name="hd2")
                   if hd2_d is not None else None)
            qt = (cpool.tile([D, Tqpad - qA], f32r, name="qt")
                  if qt_d is not None else None)
            kt = (cpool.tile([D, (nb_tot - bA) * KB], f32r, name="kt")
                  if kt_d is not None else None)
            vp = cpool.tile([KB, nb_tot * VW], bf16)
            th = cpool.tile([KB, thw], f32)
            warm = cpool.tile([KB, QSB], bf16)

            def ktsl(kb):
                if kb < kh:
                    return hd[:, kb * KB:(kb + 1) * KB]
                if kb < bA:
                    return hd2[:, (kb - kh) * KB:(kb - kh + 1) * KB]
                return kt[:, (kb - bA) * KB:(kb - bA + 1) * KB]

            def qtsl(i):
                if qoff[i + 1] <= qh:
                    return hd[:, kh * KB + qoff[i]:kh * KB + qoff[i + 1]]
                if qoff[i + 1] <= qA:
                    k0 = (bA - kh) * KB
                    return hd2[:, k0 + qoff[i] - qh:k0 + qoff[i + 1] - qh]
                return qt[:, qoff[i] - qA:qoff[i + 1] - qA]

            # PE warm-up: matmuls on a memset tile while input DMAs stream,
            # so the PE p-state ramps to full clock before real scores.
            nc.vector.memset(warm[:], 0.0)
            for _ in range(NWARM):
                sw = spool.tile([KB, GCOLS], f32, name="swarm", tag="s")
                nc.tensor.matmul(sw[:, 0:QSB], warm[:, 0:KB], warm[:, 0:QSB],
                                 start=True, stop=True)

            # DMA order: head chunks first, then the rest in roughly the
            # order compute consumes them.
            nc.sync.dma_start(hd[:], hd_d[:])
            if hd2_d is not None:
                nc.sync.dma_start(hd2[:], hd2_d[:])
            if DMA_ORDER == "A":
                if kt_d is not None:
                    nc.sync.dma_start(kt[:], kt_d[:])
                qM = qoff[min(5, nsb)]
                if qt_d is not None and qM > qA:
                    nc.sync.dma_start(qt[:, 0:qM - qA], qt_d[:, 0:qM - qA])
                nc.sync.dma_start(vp[:], vp_d[:])
                nc.sync.dma_start(th[:], th_d[:])
                if qt_d is not None and Tqpad > qM:
                    nc.sync.dma_start(qt[:, qM - qA:], qt_d[:, qM - qA:])
            else:
                qM = qoff[min(4, nsb)]
                if qt_d is not None and qM > qA:
                    nc.sync.dma_start(qt[:, 0:qM - qA], qt_d[:, 0:qM - qA])
                kM = (blocks[min(3, nsb - 1)] - bA) * KB
                if kt_d is not None and kM > 0:
                    nc.sync.dma_start(kt[:, 0:kM], kt_d[:, 0:kM])
                nc.sync.dma_start(th[:], th_d[:])
                nc.sync.dma_start(vp[:], vp_d[:])
                if kt_d is not None and kM < (nb_tot - bA) * KB:
                    nc.sync.dma_start(kt[:, kM:], kt_d[:, kM:])
                if qt_d is not None and Tqpad > qM:
                    nc.sync.dma_start(qt[:, qM - qA:], qt_d[:, qM - qA:])

            # (superblock, block list, is-last-group): masked (causal
            # boundary) blocks go in the EARLIEST group so the last group's
            # PV never waits on a mask - the superblock's output chain
            # starts right after its final exp call.
            items = []
            for i in range(nsb):
                nb = blocks[i]
                masked = [kb for (kb, _chi) in slots[i]]
                clean = [kb for kb in range(nb) if kb not in masked]
                order_kbs = masked + clean
                pitch_i = QSB if widths[i] <= QSB else 2 * QSB
                gmax = max(1, GCOLS // pitch_i)
                tail1 = TAIL1 and i == nsb - 1 and nb > 1
                nbb = nb - 1 if tail1 else nb
                ng = -(-nbb // gmax)
                bsz, rem = divmod(nbb, ng)
                sizes = [bsz + 1] * rem + [bsz] * (ng - rem)
                if tail1:
                    sizes.append(1)
                kb0 = 0
                for gi, sz in enumerate(sizes):
                    items.append((i, tuple(order_kbs[kb0:kb0 + sz]),
                                  gi == len(sizes) - 1))
                    kb0 += sz

            o_ps = {}
            started = set()
            pending = deque()

            maxsub = max(nsubs)

            def emit_pv(i, kbs, last, p):
                w_ = widths[i]
                pitch = QSB if w_ <= QSB else 2 * QSB
                nsub = nsubs[i]
                for j, kb in enumerate(kbs):
                    for sub in range(nsub):
                        pw = min(KB, w_ - sub * KB)
                        c0 = j * pitch + sub * KB
                        nc.tensor.matmul(
                            o_ps[i][0:pw, sub * VW:(sub + 1) * VW],
                            p[:, c0:c0 + pw],
                            vp[:, kb * VW:(kb + 1) * VW],
                            # start zeroes the whole 2KB zero-region (bank):
                            # one accumulation group per o tile
                            start=(i not in started and j == 0 and sub == 0),
                            stop=(last and j == len(kbs) - 1
                                  and sub == nsub - 1),
                        )
                started.add(i)
                if last:
                    ob = obpool.tile([KB, maxsub * VW], f32, name=f"ob{i}",
                                     tag="ob")
                    nfull = w_ // KB
                    if nfull:
                        cw = nfull * VW
                        nc.vector.tensor_copy(ob[:, 0:cw], o_ps[i][:, 0:cw])
                        nc.sync.dma_start(
                            o_d[:, ooff[i]:ooff[i] + cw], ob[:, 0:cw])
                    if nfull < nsub:
                        # trailing sub-block covers < 128 query rows: touch
                        # only the initialized partition range
                        pw = w_ - nfull * KB
                        cl = slice(nfull * VW, (nfull + 1) * VW)
                        dl = slice(ooff[i] + nfull * VW,
                                   ooff[i] + (nfull + 1) * VW)
                        nc.vector.tensor_copy(ob[0:pw, cl],
                                              o_ps[i][0:pw, cl])
                        nc.sync.dma_start(o_d[0:pw, dl], ob[0:pw, cl])
                    del o_ps[i]

            for it, (i, kbs, last) in enumerate(items):
                w_ = widths[i]
                pitch = QSB if w_ <= QSB else 2 * QSB
                if i not in o_ps:
                    o_ps[i] = opool.tile([KB, maxsub * VW], f32,
                                         name=f"oacc{i}", tag="oacc")
                sz = len(kbs)
                s = spool.tile([KB, GCOLS], f32, tag="s")
                for j, kb in enumerate(kbs):
                    nc.tensor.matmul(
                        s[:, j * pitch:j * pitch + w_],
                        ktsl(kb),
                        qtsl(i),
                        start=True, stop=True,
                    )
                # drain the PV pipeline early near the end so the final
                # output DMA chains start during the last exp calls
                pipe = (PIPE if it < len(items) - PIPE - 1
                        else max(0, len(items) - 2 - it))
                while len(pending) > pipe:
                    emit_pv(*pending.popleft())
                p = ppool.tile([KB, GCOLS], bf16)
                if w_ == pitch:
                    nc.scalar.activation(
                        p[:, 0:sz * w_], s[:, 0:sz * w_],
                        mybir.ActivationFunctionType.Exp,
                    )
                else:
                    # bank-aligned slots: strided 3D AP skips the pad cols
                    sv = s[:, 0:sz * pitch]
                    pv_ = p[:, 0:sz * pitch]
                    s3 = bass.AP(sv.tensor, sv.offset,
                                 [sv.ap[0], [pitch, sz], [1, w_]])
                    p3 = bass.AP(pv_.tensor, pv_.offset,
                                 [pv_.ap[0], [pitch, sz], [1, w_]])
                    nc.scalar.activation(
                        p3, s3, mybir.ActivationFunctionType.Exp)
                for (kb, chi) in slots[i]:
                    if kb in kbs:
                        j = kbs.index(kb)
                        col = thcol[(i, kb)]
                        nc.vector.scalar_tensor_tensor(
                            p[:, j * pitch:j * pitch + chi],
                            th[:, 0:chi],
                            th[:, col:col + 1],
                            p[:, j * pitch:j * pitch + chi],
                            op0=mybir.AluOpType.is_ge,
                            op1=mybir.AluOpType.mult,
                        )
                pending.append((i, kbs, last, p))
            while pending:
                emit_pv(*pending.popleft())

    nc.compile()
    return nc


def _get_nc(key):
    if key not in _compiled:
        _compiled[key] = _build_nc(key)
    return _compiled[key]


def _host_inputs(query, value, keys, q_mask, v_mask, scale):
    global _last_plan
    scale = np.float32(scale)
    q = np.asarray(query, np.float32)
    v = np.asarray(value, np.float32)
    k = np.asarray(keys, np.float32)
    qm = np.asarray(q_mask).astype(bool)
    vm = np.asarray(v_mask).astype(bool)

    Lqs = [np.flatnonzero(qm[b]) for b in range(B)]
    nlqs = [len(x) for x in Lqs]
    maxq = max(max(nlqs), 64)

    lives = []
    for c in range(NCORES):
        b, par = c // 2, c % 2
        lives.append(np.flatnonzero(vm[b])[par::2])

    # per-core packed keys: only those visible to some live query
    packed = []
    for c in range(NCORES):
        b = c // 2
        live = lives[c]
        ncnt = int(np.searchsorted(live, Lqs[b][-1] + 1)) if nlqs[b] else 0
        packed.append(live[:ncnt])

    # DP over superblock boundaries (64-col granularity): minimize exp work
    # = sum(blocks_i * W_i) + per-call overhead, where blocks_i is the
    # max-over-cores key-block count at the superblock's top query.
    P = -(-maxq // 64)
    nbat = [0] * (P + 1)
    for pos in range(1, P + 1):
        mx = 1
        for c in range(NCORES):
            b = c // 2
            nlq = nlqs[b]
            if nlq == 0:
                continue
            top = Lqs[b][min(pos * 64, nlq) - 1]
            cn = int(np.searchsorted(packed[c], top + 1))
            mx = max(mx, -(-cn // KB))
        nbat[pos] = mx
    OVH = 222               # act per-call overhead in column-equivalents
    INF = float("inf")
    dp = [INF] * (P + 1)
    prev = [0] * (P + 1)
    dp[0] = 0.0
    for j1 in range(1, P + 1):
        # blocks sit at bank-aligned pitches (256 cols for W<=256, 512 for
        # wider).  Widths are 128-multiples so every PV sub-block covers a
        # full 128 partitions (PSUM accumulation start/stop must agree on
        # the partition range); the final superblock may be 64 or 128.
        if j1 < P:
            if j1 % 2:
                continue
            deltas = (4, 6, 8)
        else:
            deltas = (1, 2, 4, 6, 8)
        for dlt in deltas:
            j0 = j1 - dlt
            if j0 < 0 or j0 % 2 or dp[j0] == INF:
                continue
            w_ = dlt * 64
            nb = nbat[j1]
            pitch = QSB if w_ <= QSB else 2 * QSB
            ng = -(-nb // max(1, GCOLS // pitch))
            # act cost in column-equivalents; sub-256 widths pay 4x on the
            # PE (f32r), charge a fraction of that as pipeline pressure
            cost = dp[j0] + nb * w_ + OVH * ng
            if w_ < QSB:
                cost += 0.3 * nb * w_
            if cost < dp[j1]:
                dp[j1] = cost
                prev[j1] = j0
    widths = []
    j = P
    while j > 0:
        widths.append((j - prev[j]) * 64)
        j = prev[j]
    widths = tuple(reversed(widths))
    nsb = len(widths)
    qoff = [0]
    for w_ in widths:
        qoff.append(qoff[-1] + w_)
    Tqpad = qoff[-1]

    # per-core, per-superblock base/top q_orig and key count
    base = np.full((NCORES, nsb), -1, np.int64)      # -1: no real cols
    cnt = np.zeros((NCORES, nsb), np.int64)
    for c in range(NCORES):
        b = c // 2
        Lq, nlq = Lqs[b], nlqs[b]
        for i in range(nsb):
            j0 = qoff[i]
            if j0 < nlq:
                base[c, i] = Lq[j0]
                top = Lq[min(qoff[i + 1], nlq) - 1]
                cnt[c, i] = np.searchsorted(packed[c], top + 1)
            else:
                cnt[c, i] = cnt[c, i - 1] if i else 0

    blocks = []
    for i in range(nsb):
        nb = max(1, int(max(-(-cnt[c, i] // KB) for c in range(NCORES))))
        if blocks:
            nb = max(nb, blocks[-1])
        blocks.append(nb)
    nb_tot = blocks[-1]
    npad = nb_tot * KB

    # causal-boundary slots: walk blocks from the top; a block is clean when
    # for every core all its real keys are <= that core's superblock base.
    slots = []
    for i in range(nsb):
        sl = []
        for kb in range(blocks[i] - 1, -1, -1):
            allvis = True
            chi = 0
            for c in range(NCORES):
                b = c // 2
                seg = packed[c][kb * KB:(kb + 1) * KB]
                if seg.size == 0 or base[c, i] < 0:
                    continue
                kmax = int(seg[-1])
                if kmax > base[c, i]:
                    allvis = False
                Lq_sb = Lqs[b][qoff[i]:min(qoff[i + 1], nlqs[b])]
                chi = max(chi, int(np.searchsorted(Lq_sb, kmax)))
            if chi > 0:
                sl.append((kb, chi))
            if allvis:
                break
        slots.append(tuple(reversed(sl)))
    slots = tuple(slots)
    key = (widths, tuple(blocks), slots)

    ns_tot = sum(len(s) for s in slots)
    maxW = max(widths)
    thw = maxW + max(1, ns_tot)
    nsubs = [-(-w_ // KB) for w_ in widths]
    ooff = [0]
    for ns_ in nsubs:
        ooff.append(ooff[-1] + ns_ * VW)
    bA = blocks[min(1, nsb - 1)]
    kA = bA * KB
    qA = qoff[min(2, nsb)]
    kh = blocks[0] if (HD2 and nsb > 1) else bA
    qh = qoff[1] if (HD2 and nsb > 1) else qA
    in_maps = []
    for c in range(NCORES):
        b = c // 2
        pk = packed[c]
        ncnt = len(pk)
        k_orig = np.full(npad, T, np.int64)
        k_orig[:ncnt] = pk
        kc = np.zeros((npad, D), np.float32)
        kc[:ncnt] = k[b][pk]
        vc = np.zeros((npad, VW), np.float32)
        vc[:ncnt, :D] = v[b][pk]
        vc[:ncnt, D] = 1.0
        qt = np.zeros((D, Tqpad), np.float32)
        if nlqs[b]:
            qt[:, :nlqs[b]] = (q[b][Lqs[b]] * scale).T
        kt = np.ascontiguousarray(kc.T)
        vp = np.ascontiguousarray(
            vc.reshape(nb_tot, KB, VW).transpose(1, 0, 2).reshape(KB, -1)
        ).astype(ml_dtypes.bfloat16)
        th = np.zeros((KB, thw), np.float32)
        th[:, :maxW] = np.arange(maxW, dtype=np.float32)[None, :]
        col = maxW
        for i in range(nsb):
            Lq_sb = Lqs[b][qoff[i]:min(qoff[i + 1], nlqs[b])]
            for (kb, _chi) in slots[i]:
                # threshold in column-index space: col kept iff its index
                # >= #cols with q_orig < k_orig  (q_orig >= k_orig)
                th[:, col] = np.searchsorted(
                    Lq_sb, k_orig[kb * KB:(kb + 1) * KB]).astype(np.float32)
                col += 1
        im = {"hd": np.ascontiguousarray(
                  np.concatenate([kt[:, :kh * KB], qt[:, :qh]], axis=1)),
              "vp": vp, "th": th}
        if (bA - kh) * KB + (qA - qh) > 0:
            im["hd2"] = np.ascontiguousarray(
                np.concatenate([kt[:, kh * KB:kA], qt[:, qh:qA]], axis=1))
        if Tqpad > qA:
            im["qt"] = np.ascontiguousarray(qt[:, qA:])
        if nb_tot > bA:
            im["kt"] = np.ascontiguousarray(kt[:, kA:])
        in_maps.append(im)

    _last_plan = {"Lqs": Lqs, "lives": lives, "packed": packed,
                  "blocks": blocks, "slots": slots, "nsb": nsb,
                  "widths": widths, "qoff": qoff, "ooff": ooff,
                  "nsubs": nsubs,
                  "Tqpad": Tqpad, "base": base, "cnt": cnt}
    return in_maps, key


def _host_gather(results, query, value, keys, q_mask, v_mask, scale):
    q = np.asarray(query, np.float32)
    v = np.asarray(value, np.float32)
    k = np.asarray(keys, np.float32)
    vm = np.asarray(v_mask).astype(bool)
    scale = np.float32(scale)
    plan = _last_plan
    nsb = plan["nsb"]
    widths, qoff = plan["widths"], plan["qoff"]

    out = np.zeros((B, T, D), np.float32)
    for b in range(B):
        Lq = plan["Lqs"][b]
        nlq = len(Lq)
        if nlq == 0:
            continue
        osum = results[2 * b]["o"].astype(np.float32) \
            + results[2 * b + 1]["o"].astype(np.float32)
        ooff = plan["ooff"]
        arr = np.empty((qoff[-1], VW), np.float32)
        for i in range(nsb):
            for sub in range(plan["nsubs"][i]):
                pw = min(KB, widths[i] - sub * KB)
                cols = slice(ooff[i] + sub * VW, ooff[i] + (sub + 1) * VW)
                r0 = qoff[i] + sub * KB
                arr[r0:r0 + pw] = osum[0:pw, cols]
        arr = arr[:nlq]
        l = arr[:, D]
        rows = arr[:, :D] / np.where(l > 0, l, 1.0)[:, None]
        nz = np.flatnonzero(vm[b])
        first = nz[0] if nz.size else T
        fix = np.flatnonzero(Lq < first)
        if fix.size:
            rr = Lq[fix]
            s = ((q[b, rr] @ k[b].T) * scale).astype(np.float32)
            s = s - np.float32(NEG_BIG)
            s = s.astype(np.float64)
            s -= s.max(axis=1, keepdims=True)
            p = np.exp(s)
            p /= p.sum(axis=1, keepdims=True)
            rows[fix] = (p @ v[b].astype(np.float64)).astype(np.float32)
        out[b][Lq] = rows
    return out


def kernel(**inputs):
    from concourse.bass_utils import run_bass_kernel_spmd

    in_maps, key = _host_inputs(**inputs)
    nc = _get_nc(key)
    res = run_bass_kernel_spmd(nc, in_maps, list(range(NCORES))).results
    return _host_gather(res, **inputs)
